# revision 20
# baseline (speedup 1.0000x reference)
"""Trainium2 Bass kernel for ActivatedAttention (B=4, T=2048, D=1024, f32).

  qkv = x @ W_in + b_in;  Q,K,V = split(qkv)
  Q = relu(rope(Q)); K = relu(rope(K)); V = relu(V)
  y = (Q @ K^T) @ V            # no softmax -> reassociate: y = Q @ (K^T @ V)
  out = swapaxes(group_norm(y), -2, -1).reshape(B, T, D)

Sharding: 8 cores = (batch b in 0..3) x (sequence half h in 0..1).
Each core projects its own 1024 rows, computes its partial K^T V, pair-
exchanges it with its batch sibling (AllGather + on-core add), then
computes y = Q @ KtV and the group norm for its rows.  The final
swapaxes/reshape is pure data movement, done on the host during unshard.

v5 changes vs v4 (211us traced):
- The pair exchange is the end-game bottleneck (ncfw trigger floor
  ~11.6us + ~20us per 1MB 2-rank AllGather, serialized).  A1 is
  reordered to launch it as early as possible: K' (paired halves) ->
  V half 0 -> KtV chunk 0 -> AllGather 0 -> V half 1 -> KtV chunk 1 ->
  AllGather 1.  Chunk 0's doorbell rings ~16us earlier than v4.
- DMA count halved: wk/wv/rope-table tensors are laid out so every
  load is a full [128,1024] tile (one dma_start each); the KtV partial
  leaves through ONE dma per chunk (a [128,4096] staging tile with a
  partition-block-rearranged DRAM AP), and each gathered chunk returns
  through ONE dma into a [128,8192] staging tile.  v4 paid ~0.6us of
  sync-engine issue time per dma_start, which delayed the first matmul
  and the collective doorbells.
- KtV partial evacuation runs entirely on ACT (idle during the KtV
  window); the gathered-chunk adds run on DVE (v4 used gpsimd at
  1.9us/add, which serialized phase C) at emission points where the
  collective is already complete, so no rope chain queues behind them.
- Junk filler matmuls bridge the A2->C seam (the last Q-rope tail)
  so the HAM activity monitor never re-throttles the PE to half clock
  (v4 lost ~20us to a K=4/8 window there).

v4 changes vs the 181us v3:
- Stationary-operand reuse: every LDWEIGHTS feeds 2 matmuls (the PE
  pays a ~110-cycle weight-swap bubble per weight load; v3 paid it per
  matmul -> 259ns/MM instead of 216).  Measured 216ns/MM median.
- V is group-centered right after its projection; centering commutes
  with K^T V and the pair-sum, so y = Q @ KtV' is already group-mean-
  free and the group-norm mean chain disappears (also numerically
  better: 3.6e-3 vs 4.45e-3 rel err).
- Group norm: ACT square (bf16) -> DVE group reduce -> ACT sqrt ->
  DVE reciprocal -> DVE normalize-multiply (bf16 out).
- Phase C contracts in qr-readiness order (0,4,1,5,2,6,3,7); the last
  Q-rope pair is split into t-halves.

Tricks kept from v3:
- RoPE channel permutation: W_in's Q/K columns are pre-permuted (even
  channels then odd channels) on the host so the interleaved-pair rotation
  becomes contiguous-block arithmetic; the permutation cancels in Q K^T.
- biases land via a pre-broadcast tensor add on the PSUM-evacuation pass
  (K/V) or an ACT Identity-activation per-partition bias (Q, transposed).
- Weights and x are bf16; y/KtV chain bf16; output bf16, cast on host.
"""

from contextlib import ExitStack

import ml_dtypes
import numpy as np
import concourse.bass as bass
import concourse.tile as tile
from concourse import bacc, mybir, bass_utils
from concourse.bass import _add_dep_helper

B, T, D = 4, 2048, 1024
TL = T // 2          # rows per core (sequence half)
EPS = 1e-5
THETA = 10000.0
NGROUP = 16          # groups per 512-column tile
GSIZE = 32

F32 = mybir.dt.float32
F32R = mybir.dt.float32r
BF16 = mybir.dt.bfloat16

KV_DT = BF16          # dtype of K/V activations + KtV collective
RG = [[0, 1], [2, 3], [4, 5], [6, 7]]
RELU = mybir.ActivationFunctionType.Relu
IDENT = mybir.ActivationFunctionType.Identity
SQUARE = mybir.ActivationFunctionType.Square
SQRT = mybir.ActivationFunctionType.Sqrt

C_ORDER = [0, 4, 1, 5, 2, 6, 3, 7]   # qr completion order (rope pairs)

_CACHE = {}


def _build(gn_trivial):
    nc = bacc.Bacc("TRN2", target_bir_lowering=False, debug=False, num_devices=8)

    xT = nc.dram_tensor("xT", [D, TL], BF16, kind="ExternalInput")
    wq = nc.dram_tensor("wq", [D, D], BF16, kind="ExternalInput")
    wk = nc.dram_tensor("wk", [D, D], BF16, kind="ExternalInput")
    wv = nc.dram_tensor("wv", [D, D], BF16, kind="ExternalInput")
    bq = nc.dram_tensor("bq", [128, 8], F32, kind="ExternalInput")
    bkvb = nc.dram_tensor("bkvb", [128, 2 * D], F32, kind="ExternalInput")
    cosq = nc.dram_tensor("cosq", [D // 2, TL], BF16, kind="ExternalInput")
    sinq = nc.dram_tensor("sinq", [D // 2, TL], BF16, kind="ExternalInput")
    cosk = nc.dram_tensor("cosk", [TL, D], BF16, kind="ExternalInput")  # cos|sin
    if not gn_trivial:
        gnw = nc.dram_tensor("gnw", [128, D], F32, kind="ExternalInput")
        gnb = nc.dram_tensor("gnb", [128, D], F32, kind="ExternalInput")
    out = nc.dram_tensor("out", [2 * TL, 512], BF16, kind="ExternalOutput")

    inv32 = 1.0 / GSIZE

    with tile.TileContext(nc) as tc, ExitStack() as st:
        psmall = st.enter_context(tc.tile_pool(name="small", bufs=1))
        pq = st.enter_context(tc.tile_pool(name="pq", bufs=1))
        pktv = st.enter_context(tc.tile_pool(name="pktv", bufs=1))
        pwq = st.enter_context(tc.tile_pool(name="pwq", bufs=1))
        ptabq = st.enter_context(tc.tile_pool(name="ptabq", bufs=1))
        pdram = st.enter_context(tc.tile_pool(name="pdram", bufs=1,
                                              space="DRAM"))

        bq_sb = psmall.tile([128, 8], F32, name="bq_sb")
        nc.sync.dma_start(bq_sb[:], bq[:])
        # bkvb's DMA is deferred into A1: x + wk must head the DMA queue so
        # the first projection matmuls start as early as possible
        bkvb_sb = psmall.tile([128, 2 * D], F32, name="bkvb_sb")
        if not gn_trivial:
            gnw_sb = psmall.tile([128, D], F32, name="gnw_sb")
            nc.sync.dma_start(gnw_sb[:], gnw[:])
            gnb_sb = psmall.tile([128, D], F32, name="gnb_sb")
            nc.sync.dma_start(gnb_sb[:], gnb[:])
        eps_sb = psmall.tile([128, 1], F32, name="eps_sb")
        nc.vector.memset(eps_sb[:], EPS)

        qr = [pq.tile([128, TL], KV_DT, name=f"qr{j}", tag=f"qr{j}")
              for j in range(8)]
        ktv = [pktv.tile([128, D], KV_DT, name=f"ktv{c}", tag=f"ktv{c}")
               for c in range(8)]
        wq_sb = [pwq.tile([128, D], BF16, name=f"wq{d}", tag=f"wq{d}")
                 for d in range(8)]
        tabq = [(ptabq.tile([128, TL], BF16, name=f"cq{j}", tag=f"cq{j}"),
                 ptabq.tile([128, TL], BF16, name=f"sq{j}", tag=f"sq{j}"))
                for j in range(4)]
        cc_out = [pdram.tile([2 * D, 512], KV_DT, name=f"cco{dh}",
                             tag=f"cco{dh}")
                  for dh in range(2)]
        ppart = st.enter_context(tc.tile_pool(name="ppart", bufs=1))

        with tc.tile_pool(name="pxt", bufs=1) as pxt, \
             tc.tile_pool(name="ppsA", bufs=8, space="PSUM") as ppsA:
            xt = [pxt.tile([128, TL], BF16, name=f"xt{d}", tag=f"xt{d}")
                  for d in range(8)]

            # ========= phase A1: K' proj + rope, V halves, KtV chunks + AG
            with tc.tile_pool(name="pkv", bufs=1) as pkv, \
                 tc.tile_pool(name="pwkv", bufs=8) as pwkv, \
                 tc.tile_pool(name="pkm", bufs=1) as pkm, \
                 tc.tile_pool(name="ptabk", bufs=8) as ptabk, \
                 tc.tile_pool(name="ptmpk", bufs=2) as ptmpk, \
                 tc.tile_pool(name="pvst", bufs=4) as pvst:

                kr = [pkv.tile([128, D], KV_DT, name=f"kr{t}", tag=f"kr{t}")
                      for t in range(8)]
                v_sb = [pkv.tile([128, D], KV_DT, name=f"v{t}", tag=f"v{t}")
                        for t in range(8)]
                km = [pkm.tile([128, D], KV_DT, name=f"km{t}", tag=f"km{t}")
                      for t in range(8)]

                # PE warm-up off a gpsimd memset (first engine op issued):
                # junk matmuls keep the HAM activity monitor busy while the
                # first x/wk DMAs land.
                warm = ptmpk.tile([128, 512], F32R, name="warm", tag="warm",
                                  bufs=1)
                nc.gpsimd.memset(warm[:].bitcast(F32), 0.0)
                wps = ppsA.tile([128, 512], F32, name="wps", tag="psA")
                for i in range(8):
                    nc.tensor.matmul(wps[:], warm[:, 0:128], warm[:],
                                     start=(i == 0), stop=(i == 7))

                # dummy pair-collective on junk data: absorbs the ~11.5us
                # first-data-collective ncfw setup latency while the PE is
                # still loading, so the real AllGathers start fast
                dum_in = pdram.tile([128, 64], KV_DT, name="dum_in",
                                    tag="dum_in")
                dum_out = pdram.tile([256, 64], KV_DT, name="dum_out",
                                     tag="dum_out")
                nc.sync.dma_start(dum_in[:], warm[:, 0:32].bitcast(KV_DT))
                nc.gpsimd.collective_compute(
                    "AllGather", mybir.AluOpType.bypass,
                    ins=[dum_in[:].opt()], outs=[dum_out[:].opt()],
                    replica_groups=RG)

                # DMA order: xt/wk pairwise per d-block (the d-outer K' loop
                # chases this stream), then bias + rope tables, then wv / wq /
                # Q tables (needed progressively later)
                wk_sb = [pwkv.tile([128, D], BF16, name=f"wk{d}", tag="wv")
                         for d in range(8)]
                for d in range(8):
                    nc.sync.dma_start(xt[d][:], xT[d * 128:(d + 1) * 128, :])
                    nc.sync.dma_start(wk_sb[d][:], wk[d * 128:(d + 1) * 128, :])
                nc.sync.dma_start(bkvb_sb[:], bkvb[:])
                ktab = []
                for tb in range(8):
                    ck = ptabk.tile([128, D], BF16, name=f"ck{tb}", tag="ck")
                    nc.sync.dma_start(ck[:], cosk[tb * 128:(tb + 1) * 128, :])
                    ktab.append(ck)
                wv_sb = [pwkv.tile([128, D], BF16, name=f"wvv{d}", tag="wv2")
                         for d in range(8)]
                for d in range(8):
                    nc.sync.dma_start(wv_sb[d][:], wv[d * 128:(d + 1) * 128, :])
                for d in range(8):
                    nc.sync.dma_start(wq_sb[d][:],
                                      wq[d * 128:(d + 1) * 128, :])
                for j in range(4):
                    cq, sq = tabq[j]
                    nc.sync.dma_start(cq[:], cosq[j * 128:(j + 1) * 128, :])
                    nc.sync.dma_start(sq[:], sinq[j * 128:(j + 1) * 128, :])

                # --- K' projection, d-outer in tb-groups of 2 (4 PSUM tiles
                # per group, so groups double-buffer in the 8 banks): the
                # first matmuls chase the xt/wk DMA stream instead of waiting
                # for all of it.  Each stationary x block feeds both
                # column-half matmuls; evac + rope chase per group.
                def k_rope(tb):
                    ck = ktab[tb]
                    x1 = km[tb][:, 0:512]
                    x2 = km[tb][:, 512:1024]
                    cks = ck[:, 0:512]
                    sks = ck[:, 512:1024]
                    t1 = ptmpk.tile([128, 512], KV_DT, name=f"t1k{tb}",
                                    tag="t1")
                    t2 = ptmpk.tile([128, 512], KV_DT, name=f"t2k{tb}",
                                    tag="t2")
                    nc.gpsimd.tensor_mul(t1[:], x1, sks)         # x1*sin
                    nc.vector.tensor_mul(x1, x1, cks)            # x1 = x1*cos
                    nc.vector.tensor_mul(t2[:], x2, sks)         # x2*sin
                    nc.vector.tensor_sub(x1, x1, t2[:])          # r1
                    nc.vector.tensor_mul(x2, x2, cks)            # x2 = x2*cos
                    nc.vector.tensor_add(x2, x2, t1[:])          # r2
                    nc.scalar.activation(kr[tb][:, 0:512], x1, RELU)
                    nc.scalar.activation(kr[tb][:, 512:1024], x2, RELU)

                for tg in range(4):
                    tbs = (2 * tg, 2 * tg + 1)
                    ps0 = {}
                    ps1 = {}
                    for tb in tbs:
                        ps0[tb] = ppsA.tile([128, 512], F32,
                                            name=f"psk0_{tb}", tag="psA")
                        ps1[tb] = ppsA.tile([128, 512], F32,
                                            name=f"psk1_{tb}", tag="psA")
                    for d in range(8):
                        for tb in tbs:
                            xs = xt[d][:, tb * 128:(tb + 1) * 128]
                            nc.tensor.matmul(ps0[tb][:], xs,
                                             wk_sb[d][:, 0:512],
                                             start=(d == 0), stop=(d == 7))
                            nc.tensor.matmul(ps1[tb][:], xs,
                                             wk_sb[d][:, 512:1024],
                                             start=(d == 0), stop=(d == 7))
                    for tb in tbs:
                        nc.vector.tensor_add(km[tb][:, 0:512], ps0[tb][:],
                                             bkvb_sb[:, 0:512])
                        nc.vector.tensor_add(km[tb][:, 512:1024], ps1[tb][:],
                                             bkvb_sb[:, 512:1024])
                        k_rope(tb)

                def v_proj():
                    """Project V (paired halves, R=2) + bias + relu, then
                    group-center each row-block.  Centering commutes with
                    K^T V and the pair-sum, so y = Q @ KtV' needs no
                    group-mean subtraction."""
                    for tb in range(8):
                        ps2 = ppsA.tile([128, 512], F32, name=f"psv0_{tb}",
                                        tag="psA")
                        ps3 = ppsA.tile([128, 512], F32, name=f"psv1_{tb}",
                                        tag="psA")
                        for d in range(8):
                            xs = xt[d][:, tb * 128:(tb + 1) * 128]
                            nc.tensor.matmul(ps2[:], xs, wv_sb[d][:, 0:512],
                                             start=(d == 0), stop=(d == 7))
                            nc.tensor.matmul(ps3[:], xs, wv_sb[d][:, 512:1024],
                                             start=(d == 0), stop=(d == 7))
                        nc.vector.tensor_add(ps2[:], ps2[:],
                                             bkvb_sb[:, 1024:1536])
                        nc.scalar.activation(v_sb[tb][:, 0:512], ps2[:], RELU)
                        nc.vector.tensor_add(ps3[:], ps3[:],
                                             bkvb_sb[:, 1536:2048])
                        nc.scalar.activation(v_sb[tb][:, 512:1024], ps3[:],
                                             RELU)
                        v3 = v_sb[tb][:].rearrange("p (g c) -> p g c",
                                                   g=2 * NGROUP)
                        vsum = pvst.tile([128, 2 * NGROUP], F32,
                                         name=f"vs{tb}", tag="vs")
                        nc.vector.reduce_sum(vsum[:], v3,
                                             axis=mybir.AxisListType.X)
                        nmean = pvst.tile([128, 2 * NGROUP], F32,
                                          name=f"nm{tb}", tag="nm")
                        nc.scalar.activation(nmean[:], vsum[:], IDENT,
                                             scale=-inv32)
                        nc.vector.tensor_add(
                            v3, v3,
                            nmean[:].broadcast_to([128, 2 * NGROUP, GSIZE]))

                def ktv_compute(dh):
                    """KtV partial matmuls for d2 half dh; evacuation into a
                    [128,4096] staging tile, alternating ACT/DVE so neither
                    engine's queue delays the doorbell."""
                    part = ppart.tile([128, 8 * 512], KV_DT, name=f"part{dh}",
                                      tag=f"part{dh}")
                    for d1c in range(8):
                        ps = ppsA.tile([128, 512], F32,
                                       name=f"psk2_{dh}_{d1c}", tag="psA")
                        for tb in range(8):
                            nc.tensor.matmul(
                                ps[:],
                                kr[tb][:, d1c * 128:(d1c + 1) * 128],
                                v_sb[tb][:, dh * 512:(dh + 1) * 512],
                                start=(tb == 0), stop=(tb == 7))
                        pslc = part[:, d1c * 512:(d1c + 1) * 512]
                        if d1c % 2 == 0:
                            nc.scalar.copy(pslc, ps[:])
                        else:
                            nc.vector.tensor_copy(pslc, ps[:])
                    return part

                def ktv_dispatch(dh, part, after=None):
                    """Ship a KtV partial (ONE dma, partition-block-
                    rearranged DRAM AP) and ring its AllGather.  `after`
                    serializes the dma behind a prior collective: a doorbell
                    arriving while ncfw is mid-op makes BOTH ops ~3x slower
                    (measured 7.8us alone vs 20-30us thrashed)."""
                    cc_in = pdram.tile([D, 512], KV_DT, name=f"cci{dh}",
                                       tag=f"cci{dh}")
                    dma = nc.sync.dma_start(
                        cc_in[:].rearrange("(b p) f -> p b f", b=8),
                        part[:].rearrange("p (b f) -> p b f", b=8))
                    if after is not None:
                        _add_dep_helper(dma.ins, after.ins, sync=True,
                                        reason="defer doorbell past prior AG")
                    return nc.gpsimd.collective_compute(
                        "AllGather", mybir.AluOpType.bypass,
                        ins=[cc_in[:].opt()], outs=[cc_out[dh][:].opt()],
                        replica_groups=RG)

                # V fully projected + centered before the KtV chunks so the
                # chunk matmuls never stall on the centering chain (stalls
                # there slide the AllGather doorbells, which are the
                # end-game critical path).  Chunk 1's dispatch is deferred
                # into A2, chained behind AllGather 0.
                v_proj()
                part0 = ktv_compute(0)
                ag0 = ktv_dispatch(0, part0)
                part1 = ktv_compute(1)

            # ========= phase A2: Q' proj (transposed layout) + rope =========
            with tc.tile_pool(name="pqm", bufs=1) as pqm, \
                 tc.tile_pool(name="pfet", bufs=1) as pfet, \
                 tc.tile_pool(name="ptmpq", bufs=2) as ptmpq:

                # each gathered chunk returns via ONE dma into a [128,8192]
                # staging tile (both pair partials); the on-core adds are
                # emitted after ALL rope chains (DVE) so a collective wait
                # can never head-block a rope.
                def stage_fetch(dh):
                    sg = pfet.tile([128, 16 * 512], KV_DT, name=f"stg{dh}",
                                   tag=f"stg{dh}")
                    nc.sync.dma_start(
                        sg[:].rearrange("p (b f) -> p b f", b=16),
                        cc_out[dh][:].rearrange("(b p) f -> p b f", b=16))
                    return sg

                stage = {0: stage_fetch(0)}

                def ktv_add(dh):
                    sg = stage[dh]
                    for d1c in C_ORDER:
                        nc.vector.tensor_add(
                            ktv[d1c][:, dh * 512:(dh + 1) * 512],
                            sg[:, d1c * 512:(d1c + 1) * 512],
                            sg[:, (8 + d1c) * 512:(9 + d1c) * 512])

                qm = [pqm.tile([128, TL], BF16, name=f"qm{j}", tag=f"qm{j}")
                      for j in range(8)]

                def q_proj(cp):
                    """Both t-halves of W^T x for channel block cp: one
                    stationary weight block, two matmuls."""
                    psq0 = ppsA.tile([128, 512], F32, name=f"psq0_{cp}",
                                     tag="psA")
                    psq1 = ppsA.tile([128, 512], F32, name=f"psq1_{cp}",
                                     tag="psA")
                    for d in range(8):
                        ws = wq_sb[d][:, cp * 128:(cp + 1) * 128]
                        nc.tensor.matmul(psq0[:], ws, xt[d][:, 0:512],
                                         start=(d == 0), stop=(d == 7))
                        nc.tensor.matmul(psq1[:], ws, xt[d][:, 512:1024],
                                         start=(d == 0), stop=(d == 7))
                    nc.scalar.activation(qm[cp][:, 0:512], psq0[:],
                                         IDENT, bias=bq_sb[:, cp:cp + 1])
                    nc.scalar.activation(qm[cp][:, 512:1024], psq1[:],
                                         IDENT, bias=bq_sb[:, cp:cp + 1])

                def q_rope(j, ts, use_gp=True):
                    cq, sq = tabq[j]
                    x1 = qm[j][:, ts]
                    x2 = qm[j + 4][:, ts]
                    cqs = cq[:, ts]
                    sqs = sq[:, ts]
                    n = ts.stop - ts.start
                    t1 = ptmpq.tile([128, n], BF16, name=f"t1q{j}_{ts.start}",
                                    tag="t1", bufs=1)
                    t2 = ptmpq.tile([128, n], BF16, name=f"t2q{j}_{ts.start}",
                                    tag="t2", bufs=1)
                    if use_gp:
                        nc.gpsimd.tensor_mul(t1[:], x1, sqs)     # x1*sin
                    else:
                        nc.vector.tensor_mul(t1[:], x1, sqs)
                    nc.vector.tensor_mul(x1, x1, cqs)            # x1*cos
                    nc.vector.tensor_mul(t2[:], x2, sqs)         # x2*sin
                    nc.vector.tensor_sub(x1, x1, t2[:])          # r1
                    nc.vector.tensor_mul(x2, x2, cqs)            # x2*cos
                    nc.vector.tensor_add(x2, x2, t1[:])          # r2
                    nc.scalar.activation(qr[j][:, ts], x1, RELU)
                    nc.scalar.activation(qr[j + 4][:, ts], x2, RELU)

                # pair (j, j+4) projected back-to-back so rope(j) starts while
                # the next pair is on the PE; the last pair's rope is split
                # into t-halves (DVE-only) to shorten the A2->C tail.
                for j in range(4):
                    q_proj(j)
                    q_proj(j + 4)
                    if j < 3:
                        q_rope(j, slice(0, TL))
                        if j == 0:
                            # chunk 1 ships now: the dep on ag0 keeps its
                            # doorbell out of AllGather 0's processing window,
                            # and only rope j=1's gpsimd op queues behind the
                            # trigger (which clears by then)
                            ag1 = ktv_dispatch(1, part1, after=ag0)
                            stage[1] = stage_fetch(1)
                    else:
                        q_rope(j, slice(0, 512), use_gp=False)
                        q_rope(j, slice(512, TL), use_gp=False)
                ktv_add(0)
                ktv_add(1)

                # filler: bridge the last rope tail so the HAM activity
                # monitor keeps the PE at full clock into phase C
                wps2 = ppsA.tile([128, 512], F32, name="wps2", tag="psA")
                for i in range(10):
                    nc.tensor.matmul(wps2[:], xt[0][:, 0:128], xt[0][:, 0:512],
                                     start=(i == 0), stop=(i == 9))

        # ================= phase C: y = Q' @ KtV' + fused group norm ========
        # KtV' is group-centered, so y IS the mean-subtracted activation:
        #   rstd = 1/sqrt(sum(y^2)/32 + eps);  out = y * rstd  [* gnw + gnb]
        # Row-blocks in pairs: one [128,1024] PSUM tile spans two banks.
        with tc.tile_pool(name="pgn", bufs=3) as pgn, \
             tc.tile_pool(name="pstat", bufs=4) as pstat, \
             tc.tile_pool(name="ppsY", bufs=1, space="PSUM") as ppsY:
            NG2 = 2 * NGROUP
            for th in range(2):
                for pb in range(4):
                    ps = ppsY.tile([128, 1024], F32, name=f"psy{th}_{pb}",
                                   tag=f"psy{pb}")
                    for half in range(2):
                        tb = 2 * pb + half
                        dst = ps[:, half * 512:(half + 1) * 512]
                        for c in C_ORDER:
                            nc.tensor.matmul(
                                dst, qr[c][:, tb * 128:(tb + 1) * 128],
                                ktv[c][:, th * 512:(th + 1) * 512],
                                start=(c == 0), stop=(c == 7))
                    ps4 = ps[:].rearrange("p (u g c) -> p u g c", u=2,
                                          g=NGROUP)
                    sqt = pgn.tile([128, 1024], BF16, name=f"sqt{th}_{pb}",
                                   tag="sqt")
                    nc.scalar.activation(sqt[:], ps[:], SQUARE)
                    sums2 = pstat.tile([128, NG2], F32, name=f"s2{th}_{pb}",
                                       tag="s2")
                    nc.vector.reduce_sum(
                        sums2[:].rearrange("p (u g) -> p u g", u=2),
                        sqt[:].rearrange("p (u g c) -> p u g c", u=2,
                                         g=NGROUP),
                        axis=mybir.AxisListType.X)
                    sd = pstat.tile([128, NG2], F32, name=f"sd{th}_{pb}",
                                    tag="sd")
                    nc.scalar.activation(sd[:], sums2[:], SQRT,
                                         scale=inv32, bias=eps_sb[:])
                    rstd = pstat.tile([128, NG2], F32, name=f"rs{th}_{pb}",
                                      tag="rs")
                    nc.vector.reciprocal(rstd[:], sd[:])
                    r3 = rstd[:].rearrange("p (u g) -> p u g", u=2)
                    yout = pgn.tile([128, 1024], KV_DT, name=f"yo{th}_{pb}",
                                    tag="yout")
                    yo4 = yout[:].rearrange("p (u g c) -> p u g c", u=2,
                                            g=NGROUP)
                    cs = slice(th * 512, (th + 1) * 512)
                    if gn_trivial:
                        nc.vector.tensor_mul(
                            yo4, ps4, r3.broadcast_to([128, 2, NGROUP, GSIZE]))
                    else:
                        ygn = pgn.tile([128, 1024], F32, name=f"yg{th}_{pb}",
                                       tag="ygn")
                        y4 = ygn[:].rearrange("p (u g c) -> p u g c", u=2,
                                              g=NGROUP)
                        nc.vector.tensor_mul(
                            y4, ps4, r3.broadcast_to([128, 2, NGROUP, GSIZE]))
                        for half in range(2):
                            hs = slice(half * 512, (half + 1) * 512)
                            nc.gpsimd.tensor_mul(ygn[:, hs], ygn[:, hs],
                                                 gnw_sb[:, cs])
                            nc.gpsimd.tensor_add(yout[:, hs], ygn[:, hs],
                                                 gnb_sb[:, cs])
                    for half in range(2):
                        tb = 2 * pb + half
                        ro = th * TL + tb * 128
                        nc.sync.dma_start(
                            out[ro:ro + 128, :],
                            yout[:, half * 512:(half + 1) * 512])

    nc.compile()
    return nc


def _get_nc(gn_trivial):
    key = ("nc", gn_trivial)
    if key not in _CACHE:
        _CACHE[key] = _build(gn_trivial)
    return _CACHE[key]


def _make_in_maps(x, W_in, b_in, gn_weight, gn_bias, gn_trivial):
    perm = np.concatenate([np.arange(0, D, 2), np.arange(1, D, 2)])
    wq_h = np.ascontiguousarray(W_in[:, 0:D][:, perm]).astype(
        ml_dtypes.bfloat16)
    wk_h = np.ascontiguousarray(W_in[:, D:2 * D][:, perm]).astype(
        ml_dtypes.bfloat16)
    wv_h = np.ascontiguousarray(W_in[:, 2 * D:3 * D]).astype(
        ml_dtypes.bfloat16)
    bq_h = np.ascontiguousarray(b_in[0:D][perm].reshape(8, 128).T)
    bkv_row = np.concatenate([b_in[D:2 * D][perm], b_in[2 * D:3 * D]])
    bkvb_h = np.ascontiguousarray(
        np.broadcast_to(bkv_row[None, :], (128, 2 * D))).astype(np.float32)
    gnw_h = np.ascontiguousarray(
        np.broadcast_to(np.float32(gn_weight)[None, :], (128, D)))
    gnb_h = np.ascontiguousarray(
        np.broadcast_to(np.float32(gn_bias)[None, :], (128, D)))

    inv_freq = (1.0 / (THETA ** (np.arange(0, D, 2, dtype=np.float32) / D))
                ).astype(np.float32)

    in_maps = []
    for core in range(8):
        b, h = divmod(core, 2)
        ts = np.arange(h * TL, (h + 1) * TL, dtype=np.float32)
        freqs = ts[:, None] * inv_freq[None, :]      # [TL, 512]
        cos_n = np.cos(freqs).astype(np.float32)
        sin_n = np.sin(freqs).astype(np.float32)
        xT_h = np.ascontiguousarray(
            x[b, h * TL:(h + 1) * TL, :].T).astype(ml_dtypes.bfloat16)
        cosk_h = np.ascontiguousarray(
            np.concatenate([cos_n, sin_n], axis=1)).astype(ml_dtypes.bfloat16)
        m = {
            "xT": xT_h, "wq": wq_h, "wk": wk_h, "wv": wv_h,
            "bq": bq_h, "bkvb": bkvb_h,
            "cosq": np.ascontiguousarray(cos_n.T).astype(ml_dtypes.bfloat16),
            "sinq": np.ascontiguousarray(sin_n.T).astype(ml_dtypes.bfloat16),
            "cosk": cosk_h,
        }
        if not gn_trivial:
            m["gnw"] = gnw_h
            m["gnb"] = gnb_h
        in_maps.append(m)
    return in_maps


def kernel(x, W_in, b_in, gn_weight, gn_bias, _trace=False):
    x = np.asarray(x, dtype=np.float32)
    W_in = np.asarray(W_in, dtype=np.float32)
    b_in = np.asarray(b_in, dtype=np.float32)
    gn_weight = np.asarray(gn_weight, dtype=np.float32)
    gn_bias = np.asarray(gn_bias, dtype=np.float32)

    gn_trivial = bool(np.all(gn_weight == 1.0) and np.all(gn_bias == 0.0))
    nc = _get_nc(gn_trivial)
    in_maps = _make_in_maps(x, W_in, b_in, gn_weight, gn_bias, gn_trivial)
    res = bass_utils.run_bass_kernel_spmd(nc, in_maps, core_ids=list(range(8)),
                                          trace=_trace)
    _CACHE["last_result"] = res

    outs = [np.concatenate([np.float32(res.results[i]["out"][:TL]),
                            np.float32(res.results[i]["out"][TL:])], axis=1)
            for i in range(8)]                            # [TL, D] each
    full = np.empty((B, T, D), dtype=np.float32)
    for b in range(B):
        y_gn = np.concatenate([outs[2 * b], outs[2 * b + 1]], axis=0)  # [T,D]
        full[b] = y_gn.T.reshape(T, D)
    return full
